# revision 14
# baseline (speedup 1.0000x reference)
"""Trainium2 Bass kernel for GNN NodeBlock (segment_sum + MLP), 8-core SPMD.

Strategy (degree-sorted node sharding, fp16 single-copy, dense-only):
  - Sort nodes by degree (ascending) on the host and regroup into 200
    supergroups of 500 nodes. Supergroup g goes to core g%8 at position
    g//8, so every position k holds 8 groups of near-identical max degree
    K̄_k — the SPMD program pads each node to K̄_k edge slots (rounded up
    to a multiple of 4) with little waste and needs no overflow path at
    all. Ascending order puts the smallest tiles first, so the PE starts
    within ~2µs of kernel start.
  - Edges ship once, as fp16 (rel err ~2^-11, well inside the 2e-2 gate).
    Slots are packed 4 per quad: a [128, 500] block whose partition axis
    is (slot-in-quad, feature) streams through a stationary [I32;I32;I32;
    I32], accumulating the feature-major aggregate psum[32, 500] directly.
  - Edge tiles stream in ~9 chunked DMAs (per-quad for the first two
    positions, then multi-position chunks) — few instructions keep the
    DMA engines saturated while the first matmul still starts early.
  - MLP: rhs = [node; agg] stacked [64, 500] fp16, W1 [64,32] fp16 single
    matmul (globals folded into b1 on host), Relu+bias on ACT, W2 [32,32]
    fp16 matmul, bias+cast drain on DVE. Agg psum drain on DVE. The MLP
    runs one supergroup behind the aggregation so cross-engine drains stay
    off the PE critical path. Output flushes to HBM in chunks on the ACT
    queue.
  - No collectives: cores own disjoint node sets; host permutes/gathers.
"""

import os

import numpy as np

import concourse.bacc as bacc
import concourse.bass as bass
import concourse.mybir as mybir
import concourse.tile as tile
from concourse.bass_utils import run_bass_kernel_spmd

N_NODES = 100000
N_CORES = 8
NPC = N_NODES // N_CORES  # 12500 nodes per core
P = 128
SG = 500  # nodes per supergroup
NPOS = NPC // SG  # 25 supergroup positions per core
NG = N_CORES * NPOS  # 200 groups
D = 32

_prog_cache = {}


def _host_prep(node_attr, edge_index, edge_attr, global_attr, W1, b1, W2, b2):
    E = edge_attr.shape[0]
    r = np.ascontiguousarray(edge_index[1]).astype(np.int64)

    deg = np.bincount(r, minlength=N_NODES)
    perm = np.argsort(deg, kind="stable")  # new id -> old id, degree asc
    newid = np.empty(N_NODES, dtype=np.int64)
    newid[perm] = np.arange(N_NODES, dtype=np.int64)
    degs = deg[perm]  # degree by new id (non-decreasing)

    rn = newid[r]  # receiver in new ids
    grp = rn // SG  # 0..199
    k_pos = grp // N_CORES  # supergroup position 0..24
    core = grp % N_CORES
    n_in = rn % SG  # column within supergroup

    # rank of each edge within its node
    order = np.argsort(rn, kind="stable")
    starts = np.zeros(N_NODES, dtype=np.int64)
    np.cumsum(degs[:-1], out=starts[1:])
    rank = np.empty(E, dtype=np.int64)
    rank[order] = np.arange(E, dtype=np.int64) - starts[rn[order]]

    # per-position quad count: max degree among its 8*SG nodes, / 4 rounded up
    Kbar = np.maximum(degs.reshape(NPOS, N_CORES * SG).max(axis=1), 1)
    Q = -(-Kbar // 4)
    q_off = np.zeros(NPOS + 1, dtype=np.int64)
    np.cumsum(Q * SG, out=q_off[1:])
    q_total = int(q_off[-1])

    ea16 = np.ascontiguousarray(edge_attr, dtype=np.float32).astype(np.float16)

    A = np.zeros((N_CORES, 4, D, q_total), dtype=np.float16)
    A[core, rank % 4, :, q_off[k_pos] + (rank // 4) * SG + n_in] = ea16
    A = A.reshape(N_CORES, P, q_total)

    # node features, permuted + arranged per core/position, transposed
    nodeP = np.asarray(node_attr, dtype=np.float32)[perm].astype(np.float16)
    nodeT = np.ascontiguousarray(
        nodeP.reshape(NPOS, N_CORES, SG, D).transpose(1, 3, 0, 2)
    ).reshape(N_CORES, D, NPC)

    g0 = np.asarray(global_attr, dtype=np.float32).reshape(1, D)
    W1 = np.asarray(W1, dtype=np.float32)
    b1p = (np.asarray(b1, dtype=np.float32) + (g0 @ W1[2 * D :]).reshape(-1)).reshape(
        D, 1
    )
    w1 = np.ascontiguousarray(W1[: 2 * D]).astype(np.float16)  # [64, 32]
    w2 = np.ascontiguousarray(np.asarray(W2, dtype=np.float32)).astype(np.float16)
    b2p = np.asarray(b2, dtype=np.float32).reshape(D, 1)

    ident4 = np.ascontiguousarray(np.tile(np.eye(D, dtype=np.float16), (4, 1)))

    in_maps = []
    for c in range(N_CORES):
        in_maps.append(
            {
                "edges": A[c],
                "ident4": ident4,
                "nodeT": nodeT[c],
                "w1": w1,
                "w2": w2,
                "b1p": b1p,
                "b2p": b2p,
            }
        )
    return in_maps, tuple(int(q) for q in Q), perm


def _plan_chunks(Q):
    """Group positions into DMA chunks: positions 0 and 1 alone (loaded
    per-quad for a fast PE start), then greedy chunks of <= 28 quads,
    keeping the final chunk small so the pipeline tail stays short."""
    chunks = [[0], [1]]
    cur = []
    cur_q = 0
    for s in range(2, NPOS):
        if cur and cur_q + Q[s] > 28:
            chunks.append(cur)
            cur = []
            cur_q = 0
        cur.append(s)
        cur_q += Q[s]
    if cur:
        chunks.append(cur)
    if len(chunks[-1]) > 1 and sum(Q[s] for s in chunks[-1]) > 12:
        last = chunks[-1]
        chunks[-1] = last[:-1]
        chunks.append([last[-1]])
    return chunks


def _build_program(Q):
    if Q in _prog_cache:
        return _prog_cache[Q]

    f16 = mybir.dt.float16
    f32 = mybir.dt.float32
    nc = bacc.Bacc(
        "TRN2", target_bir_lowering=False, debug=False, num_devices=N_CORES
    )

    q_off = [0]
    for q in Q:
        q_off.append(q_off[-1] + q * SG)
    q_total = q_off[-1]

    chunks = _plan_chunks(Q)
    chunk_quads = [sum(Q[s] for s in ch) for ch in chunks]
    maxw = max(chunk_quads) * SG

    edges_d = nc.dram_tensor("edges", [P, q_total], f16, kind="ExternalInput")
    ident4_d = nc.dram_tensor("ident4", [P, D], f16, kind="ExternalInput")
    nodeT_d = nc.dram_tensor("nodeT", [D, NPC], f16, kind="ExternalInput")
    w1_d = nc.dram_tensor("w1", [2 * D, D], f16, kind="ExternalInput")
    w2_d = nc.dram_tensor("w2", [D, D], f16, kind="ExternalInput")
    b1p_d = nc.dram_tensor("b1p", [D, 1], f32, kind="ExternalInput")
    b2p_d = nc.dram_tensor("b2p", [D, 1], f32, kind="ExternalInput")
    outT_d = nc.dram_tensor("outT", [D, NPC], f16, kind="ExternalOutput")

    with tile.TileContext(nc) as tc:
        with (
            tc.tile_pool(name="const", bufs=1) as cpool,
            tc.tile_pool(name="edges", bufs=3) as epool,
            tc.tile_pool(name="mlp", bufs=3) as mpool,
            tc.tile_pool(name="psA", bufs=4, space="PSUM") as pspool,
            tc.tile_pool(name="ps1", bufs=2, space="PSUM") as p1pool,
            tc.tile_pool(name="ps2", bufs=2, space="PSUM") as p2pool,
        ):
            # first chunks on the SP queue before anything else, per-quad
            # so the first matmul starts after just 128 KB
            def load_chunk(ci):
                ch = chunks[ci]
                base = q_off[ch[0]]
                width = chunk_quads[ci] * SG
                et = epool.tile([P, maxw], f16, tag="et")
                if ci < 2:
                    for j in range(chunk_quads[ci]):
                        nc.sync.dma_start(
                            out=et[:, j * SG : (j + 1) * SG],
                            in_=edges_d.ap()[:, base + j * SG : base + (j + 1) * SG],
                        )
                else:
                    nc.sync.dma_start(
                        out=et[:, :width], in_=edges_d.ap()[:, base : base + width]
                    )
                return et

            et0 = load_chunk(0)

            # consts on the ACT queue (parallel with the edge stream)
            ident4_sb = cpool.tile([P, D], f16)
            nc.scalar.dma_start(out=ident4_sb[:], in_=ident4_d.ap())
            w1_sb = cpool.tile([2 * D, D], f16)
            nc.scalar.dma_start(out=w1_sb[:], in_=w1_d.ap())
            w2_sb = cpool.tile([D, D], f16)
            nc.scalar.dma_start(out=w2_sb[:], in_=w2_d.ap())
            b1p_sb = cpool.tile([D, 1], f32)
            nc.scalar.dma_start(out=b1p_sb[:], in_=b1p_d.ap())
            b2p_sb = cpool.tile([D, 1], f32)
            nc.scalar.dma_start(out=b2p_sb[:], in_=b2p_d.ap())

            # [node(0:32); agg(32:64)] stacked MLP input, and the fp16 output
            nodeAgg = cpool.tile([2 * D, NPC], f16)
            nc.scalar.dma_start(out=nodeAgg[:D, :], in_=nodeT_d.ap())
            outb = cpool.tile([D, NPC], f16)

            hH_hist = {}

            def mlp_front(s):
                cols = slice(s * SG, (s + 1) * SG)
                ph = p1pool.tile([D, SG], f32, tag="ph")
                nc.tensor.matmul(
                    out=ph[:],
                    lhsT=w1_sb[:],
                    rhs=nodeAgg[:, cols],
                    start=True,
                    stop=True,
                )
                hH = mpool.tile([D, SG], f16, tag="hH")
                nc.scalar.activation(
                    out=hH[:],
                    in_=ph[:],
                    func=mybir.ActivationFunctionType.Relu,
                    bias=b1p_sb[:],
                    scale=1.0,
                )
                hH_hist[s] = hH

            def mlp_back(s):
                cols = slice(s * SG, (s + 1) * SG)
                hH = hH_hist.pop(s)
                po = p2pool.tile([D, SG], f32, tag="po")
                nc.tensor.matmul(
                    out=po[:], lhsT=w2_sb[:], rhs=hH[:], start=True, stop=True
                )
                nc.vector.tensor_tensor(
                    out=outb[:, cols],
                    in0=po[:],
                    in1=b2p_sb[:].to_broadcast([D, SG]),
                    op=mybir.AluOpType.add,
                )

            out_flushed = 0

            def flush_out(upto):
                nonlocal out_flushed
                if upto > out_flushed:
                    cols = slice(out_flushed * SG, upto * SG)
                    nc.scalar.dma_start(out=outT_d.ap()[:, cols], in_=outb[:, cols])
                    out_flushed = upto

            pending = [et0]
            if len(chunks) > 1:
                pending.append(load_chunk(1))
            if len(chunks) > 2:
                pending.append(load_chunk(2))
            loaded = len(pending)
            et = pending[0]
            ci = 0
            local = 0  # quad offset within current chunk
            for s in range(NPOS):
                q = Q[s]
                cols = slice(s * SG, (s + 1) * SG)
                # advance to next chunk when s crosses its boundary
                if ci + 1 < len(chunks) and s == chunks[ci + 1][0]:
                    ci += 1
                    pending.pop(0)
                    et = pending[0]
                    local = 0
                    if loaded < len(chunks):
                        pending.append(load_chunk(loaded))
                        loaded += 1

                ps = pspool.tile([D, SG], f32, tag="ps")
                for j in range(q):
                    nc.tensor.matmul(
                        out=ps[:],
                        lhsT=ident4_sb[:],
                        rhs=et[:, (local + j) * SG : (local + j + 1) * SG],
                        start=(j == 0),
                        stop=(j == q - 1),
                        skip_group_check=True,
                    )
                local += q
                # drain agg into the stacked MLP input (DVE; Pool can't read PSUM)
                nc.vector.tensor_copy(out=nodeAgg[D:, cols], in_=ps[:])

                # MLP pipelined: front of s-1, back of s-2
                if s >= 1:
                    mlp_front(s - 1)
                if s >= 2:
                    mlp_back(s - 2)
                    if (s - 2) % 5 == 4:
                        flush_out(s - 2 + 1)

            mlp_front(NPOS - 1)
            mlp_back(NPOS - 2)
            mlp_back(NPOS - 1)
            flush_out(NPOS)

    nc.finalize()
    _prog_cache[Q] = nc
    return nc


def kernel(**inputs):
    in_maps, Q, perm = _host_prep(**inputs)
    nc = _build_program(Q)
    trace = bool(os.environ.get("KERNEL_TRACE"))
    res = run_bass_kernel_spmd(nc, in_maps, list(range(N_CORES)), trace=trace)
    if trace:
        print(f"HW exec time: {res.exec_time_ns} ns")
        print(f"mean exec time: {res.mean_exec_time_ns} ns")
    out_all = np.empty((NPOS, N_CORES, SG, D), dtype=np.float32)
    for c in range(N_CORES):
        out_all[:, c] = (
            res.results[c]["outT"].astype(np.float32).T.reshape(NPOS, SG, D)
        )
    out = np.empty((N_NODES, D), dtype=np.float32)
    out[perm] = out_all.reshape(N_NODES, D)
    return out


# revision 17
# speedup vs baseline: 1.0365x; 1.0365x over previous
"""Trainium2 Bass kernel for GNN NodeBlock (segment_sum + MLP), 8-core SPMD.

Strategy (degree-sorted node sharding, fp16 single-copy, dense-only):
  - Sort nodes by degree (ascending) on the host and regroup into 200
    supergroups of 500 nodes. Supergroup g goes to core g%8 at position
    g//8, so every position k holds 8 groups of near-identical max degree
    K̄_k — the SPMD program pads each node to K̄_k edge slots (rounded up
    to a multiple of 4) with little waste and needs no overflow path at
    all. Ascending order puts the smallest tiles first, so the PE starts
    within ~2µs of kernel start.
  - Edges ship once, as fp16 (rel err ~2^-11, well inside the 2e-2 gate).
    Slots are packed 4 per quad: a [128, 500] block whose partition axis
    is (slot-in-quad, feature) streams through a stationary [I32;I32;I32;
    I32], accumulating the feature-major aggregate psum[32, 500] directly.
  - MLP: rhs = [node; agg] stacked [64, 500] fp16, W1 [64,32] fp16 single
    matmul (globals folded into b1 on host), Relu+bias on ACT, W2 [32,32]
    fp16 matmul, bias+cast drain on DVE. Agg psum drain on DVE. The MLP
    runs one supergroup behind the aggregation so cross-engine drains stay
    off the PE critical path. Output flushes to HBM in chunks on the ACT
    queue; edge tiles prefetch 6 deep on the SP queue.
  - No collectives: cores own disjoint node sets; host permutes/gathers.
"""

import os

import numpy as np

import concourse.bacc as bacc
import concourse.bass as bass
import concourse.mybir as mybir
import concourse.tile as tile
from concourse.bass_utils import run_bass_kernel_spmd

N_NODES = 100000
N_CORES = 8
NPC = N_NODES // N_CORES  # 12500 nodes per core
P = 128
SG = 500  # nodes per supergroup
NPOS = NPC // SG  # 25 supergroup positions per core
NG = N_CORES * NPOS  # 200 groups
D = 32

_prog_cache = {}


def _host_prep(node_attr, edge_index, edge_attr, global_attr, W1, b1, W2, b2):
    E = edge_attr.shape[0]
    r = np.ascontiguousarray(edge_index[1]).astype(np.int64)

    deg = np.bincount(r, minlength=N_NODES)
    perm = np.argsort(deg, kind="stable")  # new id -> old id, degree asc
    newid = np.empty(N_NODES, dtype=np.int64)
    newid[perm] = np.arange(N_NODES, dtype=np.int64)
    degs = deg[perm]  # degree by new id (non-decreasing)

    rn = newid[r]  # receiver in new ids
    grp = rn // SG  # 0..199
    k_pos = grp // N_CORES  # supergroup position 0..24
    core = grp % N_CORES
    n_in = rn % SG  # column within supergroup

    # rank of each edge within its node
    order = np.argsort(rn, kind="stable")
    starts = np.zeros(N_NODES, dtype=np.int64)
    np.cumsum(degs[:-1], out=starts[1:])
    rank = np.empty(E, dtype=np.int64)
    rank[order] = np.arange(E, dtype=np.int64) - starts[rn[order]]

    # per-position quad count: max degree among its 8*SG nodes, / 4 rounded up
    Kbar = np.maximum(degs.reshape(NPOS, N_CORES * SG).max(axis=1), 1)
    Q = -(-Kbar // 4)
    q_off = np.zeros(NPOS + 1, dtype=np.int64)
    np.cumsum(Q * SG, out=q_off[1:])
    q_total = int(q_off[-1])

    ea16 = np.ascontiguousarray(edge_attr, dtype=np.float32).astype(np.float16)

    A = np.zeros((N_CORES, 4, D, q_total), dtype=np.float16)
    A[core, rank % 4, :, q_off[k_pos] + (rank // 4) * SG + n_in] = ea16
    A = A.reshape(N_CORES, P, q_total)

    # node features, permuted + arranged per core/position, transposed
    nodeP = np.asarray(node_attr, dtype=np.float32)[perm].astype(np.float16)
    nodeT = np.ascontiguousarray(
        nodeP.reshape(NG, SG, D).reshape(NPOS, N_CORES, SG, D).transpose(1, 3, 0, 2)
    ).reshape(N_CORES, D, NPC)

    g0 = np.asarray(global_attr, dtype=np.float32).reshape(1, D)
    W1 = np.asarray(W1, dtype=np.float32)
    b1p = (np.asarray(b1, dtype=np.float32) + (g0 @ W1[2 * D :]).reshape(-1)).reshape(
        D, 1
    )
    w1 = np.ascontiguousarray(W1[: 2 * D]).astype(np.float16)  # [64, 32]
    w2 = np.ascontiguousarray(np.asarray(W2, dtype=np.float32)).astype(np.float16)
    b2p = np.asarray(b2, dtype=np.float32).reshape(D, 1)

    ident4 = np.ascontiguousarray(np.tile(np.eye(D, dtype=np.float16), (4, 1)))

    in_maps = []
    for c in range(N_CORES):
        in_maps.append(
            {
                "edges": A[c],
                "ident4": ident4,
                "nodeT": nodeT[c],
                "w1": w1,
                "w2": w2,
                "b1p": b1p,
                "b2p": b2p,
            }
        )
    return in_maps, tuple(int(q) for q in Q), perm


def _build_program(Q):
    if Q in _prog_cache:
        return _prog_cache[Q]

    f16 = mybir.dt.float16
    f32 = mybir.dt.float32
    nc = bacc.Bacc(
        "TRN2", target_bir_lowering=False, debug=False, num_devices=N_CORES
    )

    q_off = [0]
    for q in Q:
        q_off.append(q_off[-1] + q * SG)
    q_total = q_off[-1]
    Qmax = max(Q)

    edges_d = nc.dram_tensor("edges", [P, q_total], f16, kind="ExternalInput")
    ident4_d = nc.dram_tensor("ident4", [P, D], f16, kind="ExternalInput")
    nodeT_d = nc.dram_tensor("nodeT", [D, NPC], f16, kind="ExternalInput")
    w1_d = nc.dram_tensor("w1", [2 * D, D], f16, kind="ExternalInput")
    w2_d = nc.dram_tensor("w2", [D, D], f16, kind="ExternalInput")
    b1p_d = nc.dram_tensor("b1p", [D, 1], f32, kind="ExternalInput")
    b2p_d = nc.dram_tensor("b2p", [D, 1], f32, kind="ExternalInput")
    outT_d = nc.dram_tensor("outT", [D, NPC], f16, kind="ExternalOutput")

    with tile.TileContext(nc) as tc:
        with (
            tc.tile_pool(name="const", bufs=1) as cpool,
            tc.tile_pool(name="edges", bufs=9) as epool,
            tc.tile_pool(name="mlp", bufs=3) as mpool,
            tc.tile_pool(name="psA", bufs=4, space="PSUM") as pspool,
            tc.tile_pool(name="ps1", bufs=2, space="PSUM") as p1pool,
            tc.tile_pool(name="ps2", bufs=2, space="PSUM") as p2pool,
        ):
            # Edge tiles for positions 1-4 are issued BEFORE position 0 on
            # the SP queue: the PE's first matmul waits for tile 0, so when
            # it lands the PE starts with a 4-tile backlog, never starves,
            # and ramps to the max p-state clock within ~3us.
            ets = {}

            def load_tile(s):
                et = epool.tile([P, Qmax * SG], f16, tag="et")
                nc.sync.dma_start(
                    out=et[:, : Q[s] * SG],
                    in_=edges_d.ap()[:, q_off[s] : q_off[s] + Q[s] * SG],
                )
                ets[s] = et

            for s in (1, 2, 3, 4, 0):
                load_tile(s)

            # consts on the ACT queue (parallel with the edge stream)
            ident4_sb = cpool.tile([P, D], f16)
            nc.scalar.dma_start(out=ident4_sb[:], in_=ident4_d.ap())
            w1_sb = cpool.tile([2 * D, D], f16)
            nc.scalar.dma_start(out=w1_sb[:], in_=w1_d.ap())
            w2_sb = cpool.tile([D, D], f16)
            nc.scalar.dma_start(out=w2_sb[:], in_=w2_d.ap())
            b1p_sb = cpool.tile([D, 1], f32)
            nc.scalar.dma_start(out=b1p_sb[:], in_=b1p_d.ap())
            b2p_sb = cpool.tile([D, 1], f32)
            nc.scalar.dma_start(out=b2p_sb[:], in_=b2p_d.ap())

            # [node(0:32); agg(32:64)] stacked MLP input, and the fp16 output
            nodeAgg = cpool.tile([2 * D, NPC], f16)
            nc.scalar.dma_start(out=nodeAgg[:D, :], in_=nodeT_d.ap())
            outb = cpool.tile([D, NPC], f16)

            hH_hist = {}

            def mlp_front(s):
                cols = slice(s * SG, (s + 1) * SG)
                ph = p1pool.tile([D, SG], f32, tag="ph")
                nc.tensor.matmul(
                    out=ph[:],
                    lhsT=w1_sb[:],
                    rhs=nodeAgg[:, cols],
                    start=True,
                    stop=True,
                )
                hH = mpool.tile([D, SG], f16, tag="hH")
                nc.scalar.activation(
                    out=hH[:],
                    in_=ph[:],
                    func=mybir.ActivationFunctionType.Relu,
                    bias=b1p_sb[:],
                    scale=1.0,
                )
                hH_hist[s] = hH

            def mlp_back(s):
                cols = slice(s * SG, (s + 1) * SG)
                hH = hH_hist.pop(s)
                po = p2pool.tile([D, SG], f32, tag="po")
                nc.tensor.matmul(
                    out=po[:], lhsT=w2_sb[:], rhs=hH[:], start=True, stop=True
                )
                nc.vector.tensor_tensor(
                    out=outb[:, cols],
                    in0=po[:],
                    in1=b2p_sb[:].to_broadcast([D, SG]),
                    op=mybir.AluOpType.add,
                )

            out_flushed = 0

            def flush_out(upto):
                nonlocal out_flushed
                if upto > out_flushed:
                    cols = slice(out_flushed * SG, upto * SG)
                    nc.scalar.dma_start(out=outT_d.ap()[:, cols], in_=outb[:, cols])
                    out_flushed = upto

            for s in range(NPOS):
                q = Q[s]
                cols = slice(s * SG, (s + 1) * SG)

                if s + 5 < NPOS:
                    load_tile(s + 5)
                et = ets.pop(s)

                ps = pspool.tile([D, SG], f32, tag="ps")
                for j in range(q):
                    nc.tensor.matmul(
                        out=ps[:],
                        lhsT=ident4_sb[:],
                        rhs=et[:, j * SG : (j + 1) * SG],
                        start=(j == 0),
                        stop=(j == q - 1),
                        skip_group_check=True,
                    )
                # drain agg into the stacked MLP input (DVE; Pool can't read PSUM)
                nc.vector.tensor_copy(out=nodeAgg[D:, cols], in_=ps[:])

                # MLP pipelined: front of s-1, back of s-2
                if s >= 1:
                    mlp_front(s - 1)
                if s >= 2:
                    mlp_back(s - 2)
                    if (s - 2) % 5 == 4:
                        flush_out(s - 2 + 1)

            mlp_front(NPOS - 1)
            mlp_back(NPOS - 2)
            mlp_back(NPOS - 1)
            flush_out(NPOS)

    nc.finalize()
    _prog_cache[Q] = nc
    return nc


def kernel(**inputs):
    in_maps, Q, perm = _host_prep(**inputs)
    nc = _build_program(Q)
    trace = bool(os.environ.get("KERNEL_TRACE"))
    res = run_bass_kernel_spmd(nc, in_maps, list(range(N_CORES)), trace=trace)
    if trace:
        print(f"HW exec time: {res.exec_time_ns} ns")
        print(f"mean exec time: {res.mean_exec_time_ns} ns")
    out_all = np.empty((NPOS, N_CORES, SG, D), dtype=np.float32)
    for c in range(N_CORES):
        out_all[:, c] = (
            res.results[c]["outT"].astype(np.float32).T.reshape(NPOS, SG, D)
        )
    out = np.empty((N_NODES, D), dtype=np.float32)
    out[perm] = out_all.reshape(N_NODES, D)
    return out


# revision 19
# speedup vs baseline: 1.1595x; 1.1186x over previous
"""Trainium2 Bass kernel for GNN NodeBlock (segment_sum + MLP), 8-core SPMD.

Strategy (degree-sorted node sharding, fp16 single-copy, dense-only):
  - Sort nodes by degree (ascending) on the host and regroup into 200
    supergroups of 500 nodes. Supergroup g goes to core g%8 at position
    g//8, so every position k holds 8 groups of near-identical max degree
    K̄_k — the SPMD program pads each node to K̄_k edge slots (rounded up
    to a multiple of 4) with little waste and needs no overflow path at
    all. Ascending order puts the smallest tiles first, so the PE starts
    within ~2µs of kernel start.
  - Edges ship once, as fp16 (rel err ~2^-11, well inside the 2e-2 gate).
    Slots are packed 4 per quad: a [128, 500] block whose partition axis
    is (slot-in-quad, feature) streams through a stationary [I32;I32;I32;
    I32], accumulating the feature-major aggregate psum[32, 500] directly.
  - MLP: rhs = [node; agg] stacked [64, 500] fp16, W1 [64,32] fp16 single
    matmul (globals folded into b1 on host), Relu+bias on ACT, W2 [32,32]
    fp16 matmul, bias+cast drain on DVE. Agg psum drain on DVE. The MLP
    runs one supergroup behind the aggregation so cross-engine drains stay
    off the PE critical path. Output flushes to HBM in chunks on the ACT
    queue; edge tiles prefetch 6 deep on the SP queue.
  - No collectives: cores own disjoint node sets; host permutes/gathers.
"""

import os

import numpy as np

import concourse.bacc as bacc
import concourse.bass as bass
import concourse.mybir as mybir
import concourse.tile as tile
from concourse.bass_utils import run_bass_kernel_spmd

N_NODES = 100000
N_CORES = 8
NPC = N_NODES // N_CORES  # 12500 nodes per core
P = 128
SG = 500  # nodes per supergroup
NPOS = NPC // SG  # 25 supergroup positions per core
NG = N_CORES * NPOS  # 200 groups
D = 32

_prog_cache = {}


def _host_prep(node_attr, edge_index, edge_attr, global_attr, W1, b1, W2, b2):
    E = edge_attr.shape[0]
    r = np.ascontiguousarray(edge_index[1]).astype(np.int64)

    deg = np.bincount(r, minlength=N_NODES)
    perm = np.argsort(deg, kind="stable")  # new id -> old id, degree asc
    newid = np.empty(N_NODES, dtype=np.int64)
    newid[perm] = np.arange(N_NODES, dtype=np.int64)
    degs = deg[perm]  # degree by new id (non-decreasing)

    rn = newid[r]  # receiver in new ids
    grp = rn // SG  # 0..199
    k_pos = grp // N_CORES  # supergroup position 0..24
    core = grp % N_CORES
    n_in = rn % SG  # column within supergroup

    # rank of each edge within its node
    order = np.argsort(rn, kind="stable")
    starts = np.zeros(N_NODES, dtype=np.int64)
    np.cumsum(degs[:-1], out=starts[1:])
    rank = np.empty(E, dtype=np.int64)
    rank[order] = np.arange(E, dtype=np.int64) - starts[rn[order]]

    # per-position quad count: max degree among its 8*SG nodes, / 4 rounded up
    Kbar = np.maximum(degs.reshape(NPOS, N_CORES * SG).max(axis=1), 1)
    Q = -(-Kbar // 4)
    q_off = np.zeros(NPOS + 1, dtype=np.int64)
    np.cumsum(Q * SG, out=q_off[1:])
    q_total = int(q_off[-1])

    ea16 = np.ascontiguousarray(edge_attr, dtype=np.float32).astype(np.float16)

    A = np.zeros((N_CORES, 4, D, q_total), dtype=np.float16)
    A[core, rank % 4, :, q_off[k_pos] + (rank // 4) * SG + n_in] = ea16
    A = A.reshape(N_CORES, P, q_total)

    # node features, permuted + arranged per core/position, transposed
    nodeP = np.asarray(node_attr, dtype=np.float32)[perm].astype(np.float16)
    nodeT = np.ascontiguousarray(
        nodeP.reshape(NG, SG, D).reshape(NPOS, N_CORES, SG, D).transpose(1, 3, 0, 2)
    ).reshape(N_CORES, D, NPC)

    g0 = np.asarray(global_attr, dtype=np.float32).reshape(1, D)
    W1 = np.asarray(W1, dtype=np.float32)
    b1p = (np.asarray(b1, dtype=np.float32) + (g0 @ W1[2 * D :]).reshape(-1)).reshape(
        D, 1
    )
    w1 = np.ascontiguousarray(W1[: 2 * D]).astype(np.float16)  # [64, 32]
    w2 = np.ascontiguousarray(np.asarray(W2, dtype=np.float32)).astype(np.float16)
    b2p = np.asarray(b2, dtype=np.float32).reshape(D, 1)

    ident4 = np.ascontiguousarray(np.tile(np.eye(D, dtype=np.float16), (4, 1)))

    in_maps = []
    for c in range(N_CORES):
        in_maps.append(
            {
                "edges": A[c],
                "ident4": ident4,
                "nodeT": nodeT[c],
                "w1": w1,
                "w2": w2,
                "b1p": b1p,
                "b2p": b2p,
            }
        )
    return in_maps, tuple(int(q) for q in Q), perm


def _build_program(Q):
    if Q in _prog_cache:
        return _prog_cache[Q]

    f16 = mybir.dt.float16
    f32 = mybir.dt.float32
    nc = bacc.Bacc(
        "TRN2", target_bir_lowering=False, debug=False, num_devices=N_CORES
    )

    q_off = [0]
    for q in Q:
        q_off.append(q_off[-1] + q * SG)
    q_total = q_off[-1]
    Qmax = max(Q)

    edges_d = nc.dram_tensor("edges", [P, q_total], f16, kind="ExternalInput")
    ident4_d = nc.dram_tensor("ident4", [P, D], f16, kind="ExternalInput")
    nodeT_d = nc.dram_tensor("nodeT", [D, NPC], f16, kind="ExternalInput")
    w1_d = nc.dram_tensor("w1", [2 * D, D], f16, kind="ExternalInput")
    w2_d = nc.dram_tensor("w2", [D, D], f16, kind="ExternalInput")
    b1p_d = nc.dram_tensor("b1p", [D, 1], f32, kind="ExternalInput")
    b2p_d = nc.dram_tensor("b2p", [D, 1], f32, kind="ExternalInput")
    outT_d = nc.dram_tensor("outT", [D, NPC], f16, kind="ExternalOutput")

    with tile.TileContext(nc) as tc:
        with (
            tc.tile_pool(name="const", bufs=1) as cpool,
            tc.tile_pool(name="edges", bufs=9) as epool,
            tc.tile_pool(name="mlp", bufs=3) as mpool,
            tc.tile_pool(name="psA", bufs=4, space="PSUM") as pspool,
            tc.tile_pool(name="ps1", bufs=2, space="PSUM") as p1pool,
            tc.tile_pool(name="ps2", bufs=2, space="PSUM") as p2pool,
        ):
            # Edge tiles stream on the SP queue; tile 0 first so the PE
            # starts early, with 5 tiles in flight so it rarely starves
            # (PE gaps reset the clock-ramp and must be avoided).
            ets = {}

            def load_tile(s):
                et = epool.tile([P, Qmax * SG], f16, tag="et")
                nc.sync.dma_start(
                    out=et[:, : Q[s] * SG],
                    in_=edges_d.ap()[:, q_off[s] : q_off[s] + Q[s] * SG],
                )
                ets[s] = et

            for s in (0, 1, 2, 3, 4):
                load_tile(s)

            # consts on the ACT queue (parallel with the edge stream)
            ident4_sb = cpool.tile([P, D], f16)
            nc.scalar.dma_start(out=ident4_sb[:], in_=ident4_d.ap())
            w1_sb = cpool.tile([2 * D, D], f16)
            nc.scalar.dma_start(out=w1_sb[:], in_=w1_d.ap())
            w2_sb = cpool.tile([D, D], f16)
            nc.scalar.dma_start(out=w2_sb[:], in_=w2_d.ap())
            b1p_sb = cpool.tile([D, 1], f32)
            nc.scalar.dma_start(out=b1p_sb[:], in_=b1p_d.ap())
            b2p_sb = cpool.tile([D, 1], f32)
            nc.scalar.dma_start(out=b2p_sb[:], in_=b2p_d.ap())

            # [node(0:32); agg(32:64)] stacked MLP input, and the fp16 output
            nodeAgg = cpool.tile([2 * D, NPC], f16)
            nc.scalar.dma_start(out=nodeAgg[:D, :], in_=nodeT_d.ap())
            outb = cpool.tile([D, NPC], f16)

            hH_hist = {}

            def mlp_front(s):
                cols = slice(s * SG, (s + 1) * SG)
                ph = p1pool.tile([D, SG], f32, tag="ph")
                nc.tensor.matmul(
                    out=ph[:],
                    lhsT=w1_sb[:],
                    rhs=nodeAgg[:, cols],
                    start=True,
                    stop=True,
                )
                hH = mpool.tile([D, SG], f16, tag="hH")
                nc.scalar.activation(
                    out=hH[:],
                    in_=ph[:],
                    func=mybir.ActivationFunctionType.Relu,
                    bias=b1p_sb[:],
                    scale=1.0,
                )
                hH_hist[s] = hH

            def mlp_back(s):
                cols = slice(s * SG, (s + 1) * SG)
                hH = hH_hist.pop(s)
                po = p2pool.tile([D, SG], f32, tag="po")
                nc.tensor.matmul(
                    out=po[:], lhsT=w2_sb[:], rhs=hH[:], start=True, stop=True
                )
                # bias-add drain on ACT (Identity shares relu's table set);
                # DVE keeps only the agg CAST so mlp_front never waits on it
                nc.scalar.activation(
                    out=outb[:, cols],
                    in_=po[:],
                    func=mybir.ActivationFunctionType.Identity,
                    bias=b2p_sb[:],
                    scale=1.0,
                )

            out_flushed = 0

            def flush_out(upto):
                nonlocal out_flushed
                if upto > out_flushed:
                    cols = slice(out_flushed * SG, upto * SG)
                    nc.scalar.dma_start(out=outT_d.ap()[:, cols], in_=outb[:, cols])
                    out_flushed = upto

            for s in range(NPOS):
                q = Q[s]
                cols = slice(s * SG, (s + 1) * SG)

                if s + 5 < NPOS:
                    load_tile(s + 5)
                et = ets.pop(s)

                ps = pspool.tile([D, SG], f32, tag="ps")
                for j in range(q):
                    nc.tensor.matmul(
                        out=ps[:],
                        lhsT=ident4_sb[:],
                        rhs=et[:, j * SG : (j + 1) * SG],
                        start=(j == 0),
                        stop=(j == q - 1),
                        skip_group_check=True,
                    )
                # drain agg into the stacked MLP input (DVE; Pool can't read PSUM)
                nc.vector.tensor_copy(out=nodeAgg[D:, cols], in_=ps[:])

                # MLP pipelined: front of s-1, back of s-2
                if s >= 1:
                    mlp_front(s - 1)
                if s >= 2:
                    mlp_back(s - 2)
                    if (s - 2) % 5 == 4:
                        flush_out(s - 2 + 1)

            mlp_front(NPOS - 1)
            mlp_back(NPOS - 2)
            mlp_back(NPOS - 1)
            flush_out(NPOS)

    nc.finalize()
    _prog_cache[Q] = nc
    return nc


def kernel(**inputs):
    in_maps, Q, perm = _host_prep(**inputs)
    nc = _build_program(Q)
    trace = bool(os.environ.get("KERNEL_TRACE"))
    res = run_bass_kernel_spmd(nc, in_maps, list(range(N_CORES)), trace=trace)
    if trace:
        print(f"HW exec time: {res.exec_time_ns} ns")
        print(f"mean exec time: {res.mean_exec_time_ns} ns")
    out_all = np.empty((NPOS, N_CORES, SG, D), dtype=np.float32)
    for c in range(N_CORES):
        out_all[:, c] = (
            res.results[c]["outT"].astype(np.float32).T.reshape(NPOS, SG, D)
        )
    out = np.empty((N_NODES, D), dtype=np.float32)
    out[perm] = out_all.reshape(N_NODES, D)
    return out
